# revision 24
# baseline (speedup 1.0000x reference)
"""Trainium2 Bass kernel for segment_reduce (conv -> softmax -> hard-assign
cluster means + soft adjacency), SPMD over 8 NeuronCores.

Sharding: data-parallel over image rows. Each core processes 128 rows
(131072 pixels) of the 1024x1024 image, computing partial
  adj   = s^T @ s            [156,156]  (softmax probs, bf16 on PE)
  sums  = s_ass^T @ [x,y,1]  [156,3]    (hard-assign sums + counts)
fully fused on-chip (s never touches HBM). Partials are summed on host.

Per 128-pixel tile (pixels on partitions, clusters on free dim):
  conv logits via K=18 matmul (f32, host-prepared im2col lhsT), 6 tiles
  grouped into one 2-bank PSUM tile; one batched exp (ACT) per group with
  a gap-skipping 4D access pattern; one tensor_reduce (DVE) for Z; one
  reciprocal; per-tile scalar-AP multiply -> s_norm bf16; one batched
  is_gt 0.5 -> s_ass; three accumulating matmuls (adj chunk [0:128,:],
  symmetric corner [28,28], sums via lhsT=[x,y,1]).
"""

import sys

sys.path.insert(0, "/opt/trn_rl_repo")

import numpy as np

N_CLUSTERS = 156
H = W = 1024
N_CORES = 8
ROWS_PER_CORE = H // N_CORES          # 128
CHUNK_ROWS = 16                        # image rows per SBUF im2col chunk
N_CHUNKS = ROWS_PER_CORE // CHUNK_ROWS  # 8
TILES_PER_CHUNK = CHUNK_ROWS * W // 128  # 128 pixel-tiles (128 px each)
PXW = W // 128                         # 8 column-blocks per row
# PSUM group: 6 tiles in 2 banks at offsets {0,156,312, 512,668,824}
GROUPS = [6] * 21 + [2]                # 21*6+2 = 128 tiles per chunk
G_OFF = [0, 156, 312, 512, 668, 824]

_CACHE = {}


def _build_graph():
    import concourse.bass as bass
    import concourse.bacc as bacc
    import concourse.tile as tile
    from concourse import mybir
    from contextlib import ExitStack

    f32 = mybir.dt.float32
    f32r = mybir.dt.float32r
    bf16 = mybir.dt.bfloat16
    AF = mybir.ActivationFunctionType

    nc = bacc.Bacc("TRN2")
    # xim[c, 18*m + k, u, p] = im2col row k of pixel-tile 3u+m, pixel p:
    # 3 tiles stacked on K so one [54,128]x[54,468] matmul (block-diagonal
    # weights) computes 3 convolutions at once.
    xim = nc.dram_tensor(
        "xim", (N_CHUNKS, 54, 43, 128), f32r, kind="ExternalInput"
    )
    # xy[p, r, cb, :] = (x, y, 1) of pixel (r, cb*128 + p), bf16
    xy = nc.dram_tensor("xy", (128, ROWS_PER_CORE, PXW, 3), bf16, kind="ExternalInput")
    w = nc.dram_tensor("w", (54, 3 * N_CLUSTERS), f32r, kind="ExternalInput")
    out = nc.dram_tensor("out", (160, N_CLUSTERS), f32, kind="ExternalOutput")

    with tile.TileContext(nc) as tc, ExitStack() as ctx:
        singles = ctx.enter_context(tc.tile_pool(name="singles", bufs=1))
        xpool = ctx.enter_context(tc.tile_pool(name="xpool", bufs=2))
        epool = ctx.enter_context(tc.tile_pool(name="epool", bufs=3))
        snpool = ctx.enter_context(tc.tile_pool(name="snpool", bufs=3))
        sapool = ctx.enter_context(tc.tile_pool(name="sapool", bufs=3))
        zpool = ctx.enter_context(tc.tile_pool(name="zpool", bufs=4))
        opool = ctx.enter_context(tc.tile_pool(name="opool", bufs=1))
        psc = ctx.enter_context(tc.tile_pool(name="psc", bufs=3, space="PSUM"))
        psacc = ctx.enter_context(tc.tile_pool(name="psacc", bufs=1, space="PSUM"))

        w_s = singles.tile([54, 3 * N_CLUSTERS], f32r)
        nc.sync.dma_start(out=w_s, in_=w[:, :])
        sig_scale = singles.tile([128, 1], f32)
        nc.vector.memset(sig_scale, 1.0e6)
        sig_bias = singles.tile([128, 1], f32)
        nc.vector.memset(sig_bias, -5.0e5)

        # persistent PSUM accumulators: adj rows 0:128, plus a combined
        # [sn2|xy1]^T @ [sn2|sa] matmul holding the adj corner and sums
        adj1_ps = psacc.tile([128, N_CLUSTERS], f32)
        combo_ps = psacc.tile([31, 184], f32)

        n_tiles_total = N_CHUNKS * TILES_PER_CHUNK

        xyf = xy.rearrange("p r cb ch -> p (r cb) ch")
        for c in range(N_CHUNKS):
            X = xpool.tile([54, 43, 128], f32r)
            nc.sync.dma_start(out=X, in_=xim[c])
            t0 = 0
            for g in GROUPS:
                tiles = list(range(t0, t0 + g))
                # --- conv: one 3-tile matmul per PSUM bank ---
                cps = psc.tile([128, 980], f32)
                if g == 6:
                    for h in range(2):
                        nc.tensor.matmul(
                            cps[:, 512 * h : 512 * h + 468],
                            lhsT=X[:, t0 // 3 + h, :],
                            rhs=w_s[:, :],
                            start=True,
                            stop=True,
                        )
                else:  # g == 2 tail: tiles 126,127 stacked as K=36
                    nc.tensor.matmul(
                        cps[:, 0:312],
                        lhsT=X[0:36, 42, :],
                        rhs=w_s[0:36, 0:312],
                        start=True,
                        stop=True,
                    )
                # --- batched exp (gap-skipping 4D AP on psum src) ---
                e6 = epool.tile([128, 6, N_CLUSTERS], bf16)
                nb = (g + 2) // 3  # banks used (1 for g<=3, 2 for g=6)
                src = bass.AP(
                    tensor=cps.tensor,
                    offset=cps.offset,
                    ap=[cps.ap[0], [512, nb], [156, min(g, 3)], [1, N_CLUSTERS]],
                )
                dst = e6[:, 0:g, :].rearrange("p (b u) c -> p b u c", b=nb)
                nc.scalar.activation(out=dst, in_=src, func=AF.Exp)
                # --- Z, 1/Z ---
                zg = zpool.tile([128, 6], f32)
                rg = zpool.tile([128, 6], f32)
                nc.vector.tensor_reduce(
                    out=zg[:, 0:g],
                    in_=e6[:, 0:g, :],
                    axis=mybir.AxisListType.X,
                    op=mybir.AluOpType.add,
                )
                nc.vector.reciprocal(out=rg[:, 0:g], in_=zg[:, 0:g])
                # --- normalize (per-tile scalar-AP mul) + threshold ---
                # sn6 cols 156:159 hold (x,y,1) so lhsT [sn2|xy1] is one AP
                sn6 = snpool.tile([128, 6, 160], bf16)
                nc.sync.dma_start(
                    out=sn6[:, 0:g, 156:159],
                    in_=xyf[:, c * TILES_PER_CHUNK + t0 : c * TILES_PER_CHUNK + t0 + g, :],
                )
                for j in range(g):
                    nc.vector.tensor_scalar_mul(
                        sn6[:, j, 0:N_CLUSTERS], e6[:, j, :], rg[:, j : j + 1]
                    )
                # sab = [sn[:,128:156] | s_ass] so the combined rhs is one AP
                sab = sapool.tile([128, 6, 184], bf16)
                nc.vector.tensor_copy(
                    out=sab[:, 0:g, 0:28], in_=sn6[:, 0:g, 128:156]
                )
                # s_ass = 1{s>0.5} via saturated sigmoid on ACT (DVE relief):
                # bf16 s_norm != 0.5 maps to sigmoid(+-2e2..) = exactly 0/1
                nc.scalar.activation(
                    out=sab[:, 0:g, 28:184],
                    in_=sn6[:, 0:g, 0:N_CLUSTERS],
                    func=AF.Sigmoid,
                    scale=sig_scale[:, 0:1],
                    bias=sig_bias[:, 0:1],
                )
                # --- accumulating matmuls (same-target back-to-back) ---
                for j, t in enumerate(tiles):
                    first = c == 0 and t0 == 0 and j == 0
                    last = c == N_CHUNKS - 1 and t0 + g == TILES_PER_CHUNK and j == g - 1
                    nc.tensor.matmul(
                        adj1_ps,
                        lhsT=sn6[:, j, 0:128],
                        rhs=sn6[:, j, 0:N_CLUSTERS],
                        start=first,
                        stop=last,
                    )
                for j, t in enumerate(tiles):
                    first = c == 0 and t0 == 0 and j == 0
                    last = c == N_CHUNKS - 1 and t0 + g == TILES_PER_CHUNK and j == g - 1
                    nc.tensor.matmul(
                        combo_ps,
                        lhsT=sn6[:, j, 128:159],
                        rhs=sab[:, j, :],
                        start=first,
                        stop=last,
                    )
                t0 += g

        o1 = opool.tile([128, N_CLUSTERS], f32)
        nc.vector.tensor_copy(out=o1, in_=adj1_ps)
        nc.sync.dma_start(out=out[0:128, :], in_=o1)
        o23 = opool.tile([31, 184], f32)
        nc.vector.tensor_copy(out=o23, in_=combo_ps)
        nc.sync.dma_start(out=out[128:131, :], in_=o23[28:31, 28:184])
        nc.sync.dma_start(out=out[131:159, 0:28], in_=o23[0:28, 0:28])

    nc.compile()
    return nc


def _get_graph():
    if "nc" not in _CACHE:
        _CACHE["nc"] = _build_graph()
    return _CACHE["nc"]


def _prep_inputs(inputs, conv_w, conv_b):
    img = np.asarray(inputs, np.float32).reshape(H, W, 2)
    padded = np.zeros((2, H + 2, W + 2), np.float32)
    padded[:, 1 : H + 1, 1 : W + 1] = img.transpose(2, 0, 1)
    # host-side im2col: big[i, c, k, r, col] = padded[ch, i*128 + c*16 + kr + r, kc + col]
    big = np.empty((N_CORES, N_CHUNKS, 18, CHUNK_ROWS, W), np.float32)
    for kc in range(3):
        for kr in range(3):
            for ch in range(2):
                k = kc * 6 + kr * 2 + ch
                v = padded[ch, kr : kr + H, kc : kc + W]
                big[:, :, k] = v.reshape(N_CORES, N_CHUNKS, CHUNK_ROWS, W)
    # stack 3 consecutive pixel-tiles on the contraction dim (K=54)
    bigt = big.reshape(N_CORES, N_CHUNKS, 18, TILES_PER_CHUNK, 128)
    xim54 = np.zeros((N_CORES, N_CHUNKS, 54, 43, 128), np.float32)
    for m in range(3):
        idx = np.arange(42) * 3 + m
        xim54[:, :, 18 * m : 18 * (m + 1), 0:42] = bigt[:, :, :, idx]
        if m < 2:
            xim54[:, :, 18 * m : 18 * (m + 1), 42] = bigt[:, :, :, 126 + m]
    # constant conv bias is softmax-invariant; drop it.
    # k = kc*6 + kr*2 + ch  ->  conv_w[kr, kc, ch, :]; block-diagonal x3
    w18 = np.asarray(conv_w, np.float32).transpose(1, 0, 2, 3).reshape(18, N_CLUSTERS)
    w19 = np.zeros((54, 3 * N_CLUSTERS), np.float32)
    for m in range(3):
        w19[18 * m : 18 * (m + 1), N_CLUSTERS * m : N_CLUSTERS * (m + 1)] = w18
    import ml_dtypes

    bf = ml_dtypes.bfloat16
    in_maps = []
    for i in range(N_CORES):
        r0 = i * ROWS_PER_CORE
        xyi = np.ones((128, ROWS_PER_CORE, PXW, 3), np.float32)
        xyi[:, :, :, 0:2] = (
            img[r0 : r0 + ROWS_PER_CORE]
            .reshape(ROWS_PER_CORE, PXW, 128, 2)
            .transpose(2, 0, 1, 3)
        )
        in_maps.append({"xim": xim54[i], "xy": xyi.astype(bf), "w": w19})
    return in_maps


def _postprocess(outs):
    o = np.zeros((160, N_CLUSTERS), np.float64)
    for r in outs:
        o += np.asarray(r["out"], np.float64)
    adj = np.zeros((N_CLUSTERS, N_CLUSTERS), np.float64)
    adj[0:128, :] = o[0:128]
    adj[128:156, 128:156] = o[131:159, 0:28]
    adj[128:156, 0:128] = o[0:128, 128:156].T
    sums = o[128:130].T          # [156, 2]
    counts = o[130]              # [156]
    with np.errstate(divide="ignore", invalid="ignore"):
        nodes = sums / counts[:, None]
    return nodes.astype(np.float32), adj.astype(np.float32)


def _install_ntff_hook():
    """The agent image's antenv lacks axon_hooks; synthesize it and register
    the ctypes NTFF profiling hook against the loaded libaxon_pjrt.so."""
    import sys as _sys
    import types, ctypes, contextlib

    if "antenv.axon_hooks" in _sys.modules:
        return
    mod = types.ModuleType("antenv.axon_hooks")
    state = {"hook": None}
    mod.set_axon_ntff_profile_hook = lambda h: state.__setitem__("hook", h)
    mod.get_axon_ntff_profile_hook = lambda: state["hook"]
    _sys.modules["antenv.axon_hooks"] = mod
    import antenv

    antenv.axon_hooks = mod

    so_path = "/opt/axon/libaxon_pjrt.so"
    lib = ctypes.CDLL(so_path)
    if not hasattr(lib, "axon_start_nrt_profile"):
        return
    lib.axon_start_nrt_profile.argtypes = [
        ctypes.POINTER(ctypes.c_int64),
        ctypes.c_size_t,
    ]
    lib.axon_start_nrt_profile.restype = ctypes.c_int64
    lib.axon_stop_nrt_profile.argtypes = [ctypes.c_char_p]
    lib.axon_stop_nrt_profile.restype = ctypes.c_int64

    @contextlib.contextmanager
    def _hook(output_dir, device_ids):
        import jax

        jax.devices()
        if device_ids:
            ids = (ctypes.c_int64 * len(device_ids))(*device_ids)
            rc = lib.axon_start_nrt_profile(ids, len(device_ids))
        else:
            rc = lib.axon_start_nrt_profile(None, 0)
        if rc != 0:
            raise RuntimeError(f"axon_start_nrt_profile rc={rc}")
        try:
            yield
        finally:
            n = lib.axon_stop_nrt_profile(str(output_dir).encode())
            print(f"profile: {n} file(s) written to {output_dir}", file=_sys.stderr)

    mod.set_axon_ntff_profile_hook(_hook)


def _run(inputs, conv_w, conv_b, trace=False, **trace_kwargs):
    from concourse.bass_utils import run_bass_kernel_spmd

    if trace:
        _install_ntff_hook()

    nc = _get_graph()
    in_maps = _prep_inputs(inputs, conv_w, conv_b)
    res = run_bass_kernel_spmd(
        nc, in_maps, core_ids=list(range(N_CORES)), trace=trace, **trace_kwargs
    )
    return _postprocess(res.results), res


def kernel(inputs, conv_w, conv_b):
    (nodes, adj), _ = _run(inputs, conv_w, conv_b)
    return nodes, adj


# revision 25
# speedup vs baseline: 1.2694x; 1.2694x over previous
"""Trainium2 Bass kernel for segment_reduce (conv -> softmax -> hard-assign
cluster means + soft adjacency), SPMD over 8 NeuronCores.

Sharding: data-parallel over image rows. Each core processes 128 rows
(131072 pixels) of the 1024x1024 image, computing partial
  adj   = s^T @ s            [156,156]  (softmax probs, bf16 on PE)
  sums  = s_ass^T @ [x,y,1]  [156,3]    (hard-assign sums + counts)
fully fused on-chip (s never touches HBM). Partials are summed on host.

Per 128-pixel tile (pixels on partitions, clusters on free dim):
  conv logits via K=18 matmul (f32, host-prepared im2col lhsT), 6 tiles
  grouped into one 2-bank PSUM tile; one batched exp (ACT) per group with
  a gap-skipping 4D access pattern; one tensor_reduce (DVE) for Z; one
  reciprocal; per-tile scalar-AP multiply -> s_norm bf16; one batched
  is_gt 0.5 -> s_ass; three accumulating matmuls (adj chunk [0:128,:],
  symmetric corner [28,28], sums via lhsT=[x,y,1]).
"""

import sys

sys.path.insert(0, "/opt/trn_rl_repo")

import numpy as np

N_CLUSTERS = 156
H = W = 1024
N_CORES = 8
ROWS_PER_CORE = H // N_CORES          # 128
CHUNK_ROWS = 16                        # image rows per SBUF im2col chunk
N_CHUNKS = ROWS_PER_CORE // CHUNK_ROWS  # 8
TILES_PER_CHUNK = CHUNK_ROWS * W // 128  # 128 pixel-tiles (128 px each)
PXW = W // 128                         # 8 column-blocks per row
# PSUM group: 6 tiles in 2 banks at offsets {0,156,312, 512,668,824}
GROUPS = [6] * 21 + [2]                # 21*6+2 = 128 tiles per chunk
G_OFF = [0, 156, 312, 512, 668, 824]

_CACHE = {}


def _build_graph():
    import concourse.bass as bass
    import concourse.bacc as bacc
    import concourse.tile as tile
    from concourse import mybir
    from contextlib import ExitStack

    f32 = mybir.dt.float32
    f32r = mybir.dt.float32r
    bf16 = mybir.dt.bfloat16
    AF = mybir.ActivationFunctionType

    nc = bacc.Bacc("TRN2")
    # xim[c, 18*m + k, u, p] = im2col row k of pixel-tile 3u+m, pixel p:
    # 3 tiles stacked on K so one [54,128]x[54,468] matmul (block-diagonal
    # weights) computes 3 convolutions at once.
    xim = nc.dram_tensor(
        "xim", (N_CHUNKS, 54, 43, 128), f32r, kind="ExternalInput"
    )
    # xy[p, r, cb, :] = (x, y, 1) of pixel (r, cb*128 + p), bf16
    xy = nc.dram_tensor("xy", (128, ROWS_PER_CORE, PXW, 3), bf16, kind="ExternalInput")
    w = nc.dram_tensor("w", (54, 3 * N_CLUSTERS), f32r, kind="ExternalInput")
    out = nc.dram_tensor("out", (160, N_CLUSTERS), f32, kind="ExternalOutput")

    with tile.TileContext(nc) as tc, ExitStack() as ctx:
        singles = ctx.enter_context(tc.tile_pool(name="singles", bufs=1))
        xpool = ctx.enter_context(tc.tile_pool(name="xpool", bufs=2))
        epool = ctx.enter_context(tc.tile_pool(name="epool", bufs=3))
        snpool = ctx.enter_context(tc.tile_pool(name="snpool", bufs=3))
        sapool = ctx.enter_context(tc.tile_pool(name="sapool", bufs=3))
        zpool = ctx.enter_context(tc.tile_pool(name="zpool", bufs=4))
        opool = ctx.enter_context(tc.tile_pool(name="opool", bufs=1))
        psc = ctx.enter_context(tc.tile_pool(name="psc", bufs=3, space="PSUM"))
        psacc = ctx.enter_context(tc.tile_pool(name="psacc", bufs=1, space="PSUM"))

        w_s = singles.tile([54, 3 * N_CLUSTERS], f32r)
        nc.sync.dma_start(out=w_s, in_=w[:, :])

        # persistent PSUM accumulators: adj rows 0:128, plus a combined
        # [sn2|xy1]^T @ [sn2|sa] matmul holding the adj corner and sums
        adj1_ps = psacc.tile([128, N_CLUSTERS], f32)
        combo_ps = psacc.tile([31, 184], f32)

        n_tiles_total = N_CHUNKS * TILES_PER_CHUNK

        xyf = xy.rearrange("p r cb ch -> p (r cb) ch")
        for c in range(N_CHUNKS):
            X = xpool.tile([54, 43, 128], f32r)
            nc.sync.dma_start(out=X, in_=xim[c])
            t0 = 0
            for g in GROUPS:
                tiles = list(range(t0, t0 + g))
                # --- conv: one 3-tile matmul per PSUM bank ---
                cps = psc.tile([128, 980], f32)
                if g == 6:
                    for h in range(2):
                        nc.tensor.matmul(
                            cps[:, 512 * h : 512 * h + 468],
                            lhsT=X[:, t0 // 3 + h, :],
                            rhs=w_s[:, :],
                            start=True,
                            stop=True,
                        )
                else:  # g == 2 tail: tiles 126,127 stacked as K=36
                    nc.tensor.matmul(
                        cps[:, 0:312],
                        lhsT=X[0:36, 42, :],
                        rhs=w_s[0:36, 0:312],
                        start=True,
                        stop=True,
                    )
                # --- batched exp (gap-skipping 4D AP on psum src) ---
                e6 = epool.tile([128, 6, N_CLUSTERS], bf16)
                nb = (g + 2) // 3  # banks used (1 for g<=3, 2 for g=6)
                src = bass.AP(
                    tensor=cps.tensor,
                    offset=cps.offset,
                    ap=[cps.ap[0], [512, nb], [156, min(g, 3)], [1, N_CLUSTERS]],
                )
                dst = e6[:, 0:g, :].rearrange("p (b u) c -> p b u c", b=nb)
                nc.scalar.activation(out=dst, in_=src, func=AF.Exp)
                # --- Z, 1/Z ---
                zg = zpool.tile([128, 6], f32)
                rg = zpool.tile([128, 6], f32)
                nc.vector.tensor_reduce(
                    out=zg[:, 0:g],
                    in_=e6[:, 0:g, :],
                    axis=mybir.AxisListType.X,
                    op=mybir.AluOpType.add,
                )
                nc.vector.reciprocal(out=rg[:, 0:g], in_=zg[:, 0:g])
                # --- normalize (per-tile scalar-AP mul) + threshold ---
                # sn6 cols 156:159 hold (x,y,1) so lhsT [sn2|xy1] is one AP
                sn6 = snpool.tile([128, 6, 160], bf16)
                nc.sync.dma_start(
                    out=sn6[:, 0:g, 156:159],
                    in_=xyf[:, c * TILES_PER_CHUNK + t0 : c * TILES_PER_CHUNK + t0 + g, :],
                )
                for j in range(g):
                    nc.vector.tensor_scalar_mul(
                        sn6[:, j, 0:N_CLUSTERS], e6[:, j, :], rg[:, j : j + 1]
                    )
                # sab = [sn[:,128:156] | s_ass] so the combined rhs is one AP
                sab = sapool.tile([128, 6, 184], bf16)
                nc.vector.tensor_copy(
                    out=sab[:, 0:g, 0:28], in_=sn6[:, 0:g, 128:156]
                )
                nc.vector.tensor_scalar(
                    out=sab[:, 0:g, 28:184],
                    in0=sn6[:, 0:g, 0:N_CLUSTERS],
                    scalar1=0.5,
                    scalar2=None,
                    op0=mybir.AluOpType.is_gt,
                )
                # --- accumulating matmuls (same-target back-to-back) ---
                for j, t in enumerate(tiles):
                    first = c == 0 and t0 == 0 and j == 0
                    last = c == N_CHUNKS - 1 and t0 + g == TILES_PER_CHUNK and j == g - 1
                    nc.tensor.matmul(
                        adj1_ps,
                        lhsT=sn6[:, j, 0:128],
                        rhs=sn6[:, j, 0:N_CLUSTERS],
                        start=first,
                        stop=last,
                    )
                for j, t in enumerate(tiles):
                    first = c == 0 and t0 == 0 and j == 0
                    last = c == N_CHUNKS - 1 and t0 + g == TILES_PER_CHUNK and j == g - 1
                    nc.tensor.matmul(
                        combo_ps,
                        lhsT=sn6[:, j, 128:159],
                        rhs=sab[:, j, :],
                        start=first,
                        stop=last,
                    )
                t0 += g

        o1 = opool.tile([128, N_CLUSTERS], f32)
        nc.vector.tensor_copy(out=o1, in_=adj1_ps)
        nc.sync.dma_start(out=out[0:128, :], in_=o1)
        o23 = opool.tile([31, 184], f32)
        nc.vector.tensor_copy(out=o23, in_=combo_ps)
        nc.sync.dma_start(out=out[128:131, :], in_=o23[28:31, 28:184])
        nc.sync.dma_start(out=out[131:159, 0:28], in_=o23[0:28, 0:28])

    nc.compile()
    return nc


def _get_graph():
    if "nc" not in _CACHE:
        _CACHE["nc"] = _build_graph()
    return _CACHE["nc"]


def _prep_inputs(inputs, conv_w, conv_b):
    img = np.asarray(inputs, np.float32).reshape(H, W, 2)
    padded = np.zeros((2, H + 2, W + 2), np.float32)
    padded[:, 1 : H + 1, 1 : W + 1] = img.transpose(2, 0, 1)
    # host-side im2col: big[i, c, k, r, col] = padded[ch, i*128 + c*16 + kr + r, kc + col]
    big = np.empty((N_CORES, N_CHUNKS, 18, CHUNK_ROWS, W), np.float32)
    for kc in range(3):
        for kr in range(3):
            for ch in range(2):
                k = kc * 6 + kr * 2 + ch
                v = padded[ch, kr : kr + H, kc : kc + W]
                big[:, :, k] = v.reshape(N_CORES, N_CHUNKS, CHUNK_ROWS, W)
    # stack 3 consecutive pixel-tiles on the contraction dim (K=54)
    bigt = big.reshape(N_CORES, N_CHUNKS, 18, TILES_PER_CHUNK, 128)
    xim54 = np.zeros((N_CORES, N_CHUNKS, 54, 43, 128), np.float32)
    for m in range(3):
        idx = np.arange(42) * 3 + m
        xim54[:, :, 18 * m : 18 * (m + 1), 0:42] = bigt[:, :, :, idx]
        if m < 2:
            xim54[:, :, 18 * m : 18 * (m + 1), 42] = bigt[:, :, :, 126 + m]
    # constant conv bias is softmax-invariant; drop it.
    # k = kc*6 + kr*2 + ch  ->  conv_w[kr, kc, ch, :]; block-diagonal x3
    w18 = np.asarray(conv_w, np.float32).transpose(1, 0, 2, 3).reshape(18, N_CLUSTERS)
    w19 = np.zeros((54, 3 * N_CLUSTERS), np.float32)
    for m in range(3):
        w19[18 * m : 18 * (m + 1), N_CLUSTERS * m : N_CLUSTERS * (m + 1)] = w18
    import ml_dtypes

    bf = ml_dtypes.bfloat16
    in_maps = []
    for i in range(N_CORES):
        r0 = i * ROWS_PER_CORE
        xyi = np.ones((128, ROWS_PER_CORE, PXW, 3), np.float32)
        xyi[:, :, :, 0:2] = (
            img[r0 : r0 + ROWS_PER_CORE]
            .reshape(ROWS_PER_CORE, PXW, 128, 2)
            .transpose(2, 0, 1, 3)
        )
        in_maps.append({"xim": xim54[i], "xy": xyi.astype(bf), "w": w19})
    return in_maps


def _postprocess(outs):
    o = np.zeros((160, N_CLUSTERS), np.float64)
    for r in outs:
        o += np.asarray(r["out"], np.float64)
    adj = np.zeros((N_CLUSTERS, N_CLUSTERS), np.float64)
    adj[0:128, :] = o[0:128]
    adj[128:156, 128:156] = o[131:159, 0:28]
    adj[128:156, 0:128] = o[0:128, 128:156].T
    sums = o[128:130].T          # [156, 2]
    counts = o[130]              # [156]
    with np.errstate(divide="ignore", invalid="ignore"):
        nodes = sums / counts[:, None]
    return nodes.astype(np.float32), adj.astype(np.float32)


def _install_ntff_hook():
    """The agent image's antenv lacks axon_hooks; synthesize it and register
    the ctypes NTFF profiling hook against the loaded libaxon_pjrt.so."""
    import sys as _sys
    import types, ctypes, contextlib

    if "antenv.axon_hooks" in _sys.modules:
        return
    mod = types.ModuleType("antenv.axon_hooks")
    state = {"hook": None}
    mod.set_axon_ntff_profile_hook = lambda h: state.__setitem__("hook", h)
    mod.get_axon_ntff_profile_hook = lambda: state["hook"]
    _sys.modules["antenv.axon_hooks"] = mod
    import antenv

    antenv.axon_hooks = mod

    so_path = "/opt/axon/libaxon_pjrt.so"
    lib = ctypes.CDLL(so_path)
    if not hasattr(lib, "axon_start_nrt_profile"):
        return
    lib.axon_start_nrt_profile.argtypes = [
        ctypes.POINTER(ctypes.c_int64),
        ctypes.c_size_t,
    ]
    lib.axon_start_nrt_profile.restype = ctypes.c_int64
    lib.axon_stop_nrt_profile.argtypes = [ctypes.c_char_p]
    lib.axon_stop_nrt_profile.restype = ctypes.c_int64

    @contextlib.contextmanager
    def _hook(output_dir, device_ids):
        import jax

        jax.devices()
        if device_ids:
            ids = (ctypes.c_int64 * len(device_ids))(*device_ids)
            rc = lib.axon_start_nrt_profile(ids, len(device_ids))
        else:
            rc = lib.axon_start_nrt_profile(None, 0)
        if rc != 0:
            raise RuntimeError(f"axon_start_nrt_profile rc={rc}")
        try:
            yield
        finally:
            n = lib.axon_stop_nrt_profile(str(output_dir).encode())
            print(f"profile: {n} file(s) written to {output_dir}", file=_sys.stderr)

    mod.set_axon_ntff_profile_hook(_hook)


def _run(inputs, conv_w, conv_b, trace=False, **trace_kwargs):
    from concourse.bass_utils import run_bass_kernel_spmd

    if trace:
        _install_ntff_hook()

    nc = _get_graph()
    in_maps = _prep_inputs(inputs, conv_w, conv_b)
    res = run_bass_kernel_spmd(
        nc, in_maps, core_ids=list(range(N_CORES)), trace=trace, **trace_kwargs
    )
    return _postprocess(res.results), res


def kernel(inputs, conv_w, conv_b):
    (nodes, adj), _ = _run(inputs, conv_w, conv_b)
    return nodes, adj


# revision 27
# speedup vs baseline: 1.4952x; 1.1779x over previous
"""Trainium2 Bass kernel for segment_reduce (conv -> softmax -> hard-assign
cluster means + soft adjacency), SPMD over 8 NeuronCores.

Sharding: data-parallel over image rows. Each core processes 128 rows
(131072 pixels) of the 1024x1024 image, computing partial
  adj   = s^T @ s            [156,156]  (softmax probs, bf16 on PE)
  sums  = s_ass^T @ [x,y,1]  [156,3]    (hard-assign sums + counts)
fully fused on-chip (s never touches HBM). Partials are summed on host.

Per 128-pixel tile (pixels on partitions, clusters on free dim):
  conv logits via K=18 matmul (f32, host-prepared im2col lhsT), 6 tiles
  grouped into one 2-bank PSUM tile; one batched exp (ACT) per group with
  a gap-skipping 4D access pattern; one tensor_reduce (DVE) for Z; one
  reciprocal; per-tile scalar-AP multiply -> s_norm bf16; one batched
  is_gt 0.5 -> s_ass; three accumulating matmuls (adj chunk [0:128,:],
  symmetric corner [28,28], sums via lhsT=[x,y,1]).
"""

import sys

sys.path.insert(0, "/opt/trn_rl_repo")

import numpy as np

N_CLUSTERS = 156
H = W = 1024
N_CORES = 8
ROWS_PER_CORE = H // N_CORES          # 128
CHUNK_ROWS = 16                        # image rows per SBUF im2col chunk
N_CHUNKS = ROWS_PER_CORE // CHUNK_ROWS  # 8
TILES_PER_CHUNK = CHUNK_ROWS * W // 128  # 128 pixel-tiles (128 px each)
PXW = W // 128                         # 8 column-blocks per row
# PSUM group: 6 tiles in 2 banks at offsets {0,156,312, 512,668,824}
GROUPS = [6] * 21 + [2]                # 21*6+2 = 128 tiles per chunk
G_OFF = [0, 156, 312, 512, 668, 824]

_CACHE = {}


def _build_graph():
    import concourse.bass as bass
    import concourse.bacc as bacc
    import concourse.tile as tile
    from concourse import mybir
    from contextlib import ExitStack

    f32 = mybir.dt.float32
    f32r = mybir.dt.float32r
    bf16 = mybir.dt.bfloat16
    AF = mybir.ActivationFunctionType

    nc = bacc.Bacc("TRN2")
    # xim[c, 18*m + k, u, p] = im2col row k of pixel-tile 3u+m, pixel p:
    # 3 tiles stacked on K so one [54,128]x[54,468] matmul (block-diagonal
    # weights) computes 3 convolutions at once.
    xim = nc.dram_tensor(
        "xim", (N_CHUNKS, 54, 43, 128), f32r, kind="ExternalInput"
    )
    # xy[p, r, cb, :] = (x, y, 1) of pixel (r, cb*128 + p), bf16
    xy = nc.dram_tensor("xy", (128, ROWS_PER_CORE, PXW, 3), bf16, kind="ExternalInput")
    w = nc.dram_tensor("w", (54, 3 * N_CLUSTERS), f32r, kind="ExternalInput")
    out = nc.dram_tensor("out", (160, N_CLUSTERS), f32, kind="ExternalOutput")

    with tile.TileContext(nc) as tc, ExitStack() as ctx:
        singles = ctx.enter_context(tc.tile_pool(name="singles", bufs=1))
        xpool = ctx.enter_context(tc.tile_pool(name="xpool", bufs=2))
        epool = ctx.enter_context(tc.tile_pool(name="epool", bufs=3))
        snpool = ctx.enter_context(tc.tile_pool(name="snpool", bufs=3))
        sapool = ctx.enter_context(tc.tile_pool(name="sapool", bufs=3))
        zpool = ctx.enter_context(tc.tile_pool(name="zpool", bufs=4))
        opool = ctx.enter_context(tc.tile_pool(name="opool", bufs=1))
        psc = ctx.enter_context(tc.tile_pool(name="psc", bufs=3, space="PSUM"))
        psacc = ctx.enter_context(tc.tile_pool(name="psacc", bufs=1, space="PSUM"))

        w_s = singles.tile([54, 3 * N_CLUSTERS], f32r)
        nc.sync.dma_start(out=w_s, in_=w[:, :])

        # persistent PSUM accumulators: adj rows 0:128, plus a combined
        # [sn2|xy1]^T @ [sn2|sa] matmul holding the adj corner and sums
        adj1_ps = psacc.tile([128, N_CLUSTERS], f32)
        combo_ps = psacc.tile([31, 184], f32)

        n_tiles_total = N_CHUNKS * TILES_PER_CHUNK

        xyf = xy.rearrange("p r cb ch -> p (r cb) ch")
        for c in range(N_CHUNKS):
            X = xpool.tile([54, 43, 128], f32r)
            nc.sync.dma_start(out=X, in_=xim[c])
            t0 = 0
            for g in GROUPS:
                tiles = list(range(t0, t0 + g))
                # --- conv: one 3-tile matmul per PSUM bank ---
                cps = psc.tile([128, 980], f32)
                if g == 6:
                    for h in range(2):
                        nc.tensor.matmul(
                            cps[:, 512 * h : 512 * h + 468],
                            lhsT=X[:, t0 // 3 + h, :],
                            rhs=w_s[:, :],
                            start=True,
                            stop=True,
                        )
                else:  # g == 2 tail: tiles 126,127 stacked as K=36
                    nc.tensor.matmul(
                        cps[:, 0:312],
                        lhsT=X[0:36, 42, :],
                        rhs=w_s[0:36, 0:312],
                        start=True,
                        stop=True,
                    )
                # --- batched exp (gap-skipping 4D AP on psum src) ---
                e6 = epool.tile([128, 6, N_CLUSTERS], bf16)
                nb = (g + 2) // 3  # banks used (1 for g<=3, 2 for g=6)
                src = bass.AP(
                    tensor=cps.tensor,
                    offset=cps.offset,
                    ap=[cps.ap[0], [512, nb], [156, min(g, 3)], [1, N_CLUSTERS]],
                )
                dst = e6[:, 0:g, :].rearrange("p (b u) c -> p b u c", b=nb)
                nc.scalar.activation(out=dst, in_=src, func=AF.Exp)
                # --- Z, 1/Z ---
                zg = zpool.tile([128, 6], f32)
                rg = zpool.tile([128, 6], f32)
                nc.vector.tensor_reduce(
                    out=zg[:, 0:g],
                    in_=e6[:, 0:g, :],
                    axis=mybir.AxisListType.X,
                    op=mybir.AluOpType.add,
                )
                nc.vector.reciprocal(out=rg[:, 0:g], in_=zg[:, 0:g])
                # --- normalize (per-tile scalar-AP mul) + threshold ---
                # sn6 cols 156:159 hold (x,y,1) so lhsT [sn2|xy1] is one AP
                sn6 = snpool.tile([128, 6, 160], bf16)
                nc.sync.dma_start(
                    out=sn6[:, 0:g, 156:159],
                    in_=xyf[:, c * TILES_PER_CHUNK + t0 : c * TILES_PER_CHUNK + t0 + g, :],
                )
                for j in range(g):
                    nc.vector.tensor_scalar_mul(
                        sn6[:, j, 0:N_CLUSTERS], e6[:, j, :], rg[:, j : j + 1]
                    )
                # sab = [sn[:,128:156] | s_ass] so the combined rhs is one AP
                sab = sapool.tile([128, 6, 184], bf16)
                nc.vector.tensor_copy(
                    out=sab[:, 0:g, 0:28], in_=sn6[:, 0:g, 128:156]
                )
                nc.vector.tensor_scalar(
                    out=sab[:, 0:g, 28:184],
                    in0=sn6[:, 0:g, 0:N_CLUSTERS],
                    scalar1=0.5,
                    scalar2=None,
                    op0=mybir.AluOpType.is_gt,
                )
                # --- accumulating matmuls (same-target back-to-back) ---
                for j, t in enumerate(tiles):
                    first = c == 0 and t0 == 0 and j == 0
                    last = c == N_CHUNKS - 1 and t0 + g == TILES_PER_CHUNK and j == g - 1
                    nc.tensor.matmul(
                        adj1_ps,
                        lhsT=sn6[:, j, 0:128],
                        rhs=sn6[:, j, 0:N_CLUSTERS],
                        start=first,
                        stop=last,
                    )
                for j, t in enumerate(tiles):
                    first = c == 0 and t0 == 0 and j == 0
                    last = c == N_CHUNKS - 1 and t0 + g == TILES_PER_CHUNK and j == g - 1
                    nc.tensor.matmul(
                        combo_ps,
                        lhsT=sn6[:, j, 128:159],
                        rhs=sab[:, j, :],
                        start=first,
                        stop=last,
                    )
                t0 += g

        o1 = opool.tile([128, N_CLUSTERS], f32)
        nc.vector.tensor_copy(out=o1, in_=adj1_ps)
        nc.sync.dma_start(out=out[0:128, :], in_=o1)
        o23 = opool.tile([31, 184], f32)
        nc.vector.tensor_copy(out=o23, in_=combo_ps)
        nc.sync.dma_start(out=out[128:131, :], in_=o23[28:31, 28:184])
        nc.sync.dma_start(out=out[131:159, 0:28], in_=o23[0:28, 0:28])

    nc.compile()
    return nc


def _get_graph():
    if "nc" not in _CACHE:
        _CACHE["nc"] = _build_graph()
    return _CACHE["nc"]


def _prep_inputs(inputs, conv_w, conv_b):
    img = np.asarray(inputs, np.float32).reshape(H, W, 2)
    padded = np.zeros((2, H + 2, W + 2), np.float32)
    padded[:, 1 : H + 1, 1 : W + 1] = img.transpose(2, 0, 1)
    # host-side im2col: big[i, c, k, r, col] = padded[ch, i*128 + c*16 + kr + r, kc + col]
    big = np.empty((N_CORES, N_CHUNKS, 18, CHUNK_ROWS, W), np.float32)
    for kc in range(3):
        for kr in range(3):
            for ch in range(2):
                k = kc * 6 + kr * 2 + ch
                v = padded[ch, kr : kr + H, kc : kc + W]
                big[:, :, k] = v.reshape(N_CORES, N_CHUNKS, CHUNK_ROWS, W)
    # stack 3 consecutive pixel-tiles on the contraction dim (K=54)
    bigt = big.reshape(N_CORES, N_CHUNKS, 18, TILES_PER_CHUNK, 128)
    xim54 = np.zeros((N_CORES, N_CHUNKS, 54, 43, 128), np.float32)
    for m in range(3):
        idx = np.arange(42) * 3 + m
        xim54[:, :, 18 * m : 18 * (m + 1), 0:42] = bigt[:, :, :, idx]
        if m < 2:
            xim54[:, :, 18 * m : 18 * (m + 1), 42] = bigt[:, :, :, 126 + m]
    # constant conv bias is softmax-invariant; drop it.
    # k = kc*6 + kr*2 + ch  ->  conv_w[kr, kc, ch, :]; block-diagonal x3
    w18 = np.asarray(conv_w, np.float32).transpose(1, 0, 2, 3).reshape(18, N_CLUSTERS)
    w19 = np.zeros((54, 3 * N_CLUSTERS), np.float32)
    for m in range(3):
        w19[18 * m : 18 * (m + 1), N_CLUSTERS * m : N_CLUSTERS * (m + 1)] = w18
    import ml_dtypes

    bf = ml_dtypes.bfloat16
    in_maps = []
    for i in range(N_CORES):
        r0 = i * ROWS_PER_CORE
        xyi = np.ones((128, ROWS_PER_CORE, PXW, 3), np.float32)
        xyi[:, :, :, 0:2] = (
            img[r0 : r0 + ROWS_PER_CORE]
            .reshape(ROWS_PER_CORE, PXW, 128, 2)
            .transpose(2, 0, 1, 3)
        )
        in_maps.append({"xim": xim54[i], "xy": xyi.astype(bf), "w": w19})
    return in_maps


def _postprocess(outs):
    o = np.zeros((160, N_CLUSTERS), np.float64)
    for r in outs:
        o += np.asarray(r["out"], np.float64)
    adj = np.zeros((N_CLUSTERS, N_CLUSTERS), np.float64)
    adj[0:128, :] = o[0:128]
    adj[128:156, 128:156] = o[131:159, 0:28]
    adj[128:156, 0:128] = o[0:128, 128:156].T
    sums = o[128:130].T          # [156, 2]
    counts = o[130]              # [156]
    with np.errstate(divide="ignore", invalid="ignore"):
        nodes = sums / counts[:, None]
    return nodes.astype(np.float32), adj.astype(np.float32)


def _install_ntff_hook():
    """The agent image's antenv lacks axon_hooks; synthesize it and register
    the ctypes NTFF profiling hook against the loaded libaxon_pjrt.so."""
    import sys as _sys
    import types, ctypes, contextlib

    if "antenv.axon_hooks" in _sys.modules:
        return
    mod = types.ModuleType("antenv.axon_hooks")
    state = {"hook": None}
    mod.set_axon_ntff_profile_hook = lambda h: state.__setitem__("hook", h)
    mod.get_axon_ntff_profile_hook = lambda: state["hook"]
    _sys.modules["antenv.axon_hooks"] = mod
    import antenv

    antenv.axon_hooks = mod

    so_path = "/opt/axon/libaxon_pjrt.so"
    lib = ctypes.CDLL(so_path)
    if not hasattr(lib, "axon_start_nrt_profile"):
        return
    lib.axon_start_nrt_profile.argtypes = [
        ctypes.POINTER(ctypes.c_int64),
        ctypes.c_size_t,
    ]
    lib.axon_start_nrt_profile.restype = ctypes.c_int64
    lib.axon_stop_nrt_profile.argtypes = [ctypes.c_char_p]
    lib.axon_stop_nrt_profile.restype = ctypes.c_int64

    @contextlib.contextmanager
    def _hook(output_dir, device_ids):
        import jax

        jax.devices()
        if device_ids:
            ids = (ctypes.c_int64 * len(device_ids))(*device_ids)
            rc = lib.axon_start_nrt_profile(ids, len(device_ids))
        else:
            rc = lib.axon_start_nrt_profile(None, 0)
        if rc != 0:
            raise RuntimeError(f"axon_start_nrt_profile rc={rc}")
        try:
            yield
        finally:
            n = lib.axon_stop_nrt_profile(str(output_dir).encode())
            print(f"profile: {n} file(s) written to {output_dir}", file=_sys.stderr)

    mod.set_axon_ntff_profile_hook(_hook)


def _run(inputs, conv_w, conv_b, trace=False, **trace_kwargs):
    from concourse.bass_utils import run_bass_kernel_spmd

    if trace:
        _install_ntff_hook()

    nc = _get_graph()
    in_maps = _prep_inputs(inputs, conv_w, conv_b)
    res = run_bass_kernel_spmd(
        nc, in_maps, core_ids=list(range(N_CORES)), trace=trace, **trace_kwargs
    )
    return _postprocess(res.results), res


def kernel(inputs, conv_w, conv_b):
    (nodes, adj), _ = _run(inputs, conv_w, conv_b)
    return nodes, adj


# revision 28
# speedup vs baseline: 1.5033x; 1.0054x over previous
"""Trainium2 Bass kernel for segment_reduce (conv -> softmax -> hard-assign
cluster means + soft adjacency), SPMD over 8 NeuronCores.

Sharding: data-parallel over image rows. Each core processes 128 rows
(131072 pixels) of the 1024x1024 image, computing partial
  adj   = s^T @ s            [156,156]  (softmax probs, bf16 on PE)
  sums  = s_ass^T @ [x,y,1]  [156,3]    (hard-assign sums + counts)
fully fused on-chip (s never touches HBM). Partials are summed on host.

Per 128-pixel tile (pixels on partitions, clusters on free dim):
  conv logits via K=18 matmul (f32, host-prepared im2col lhsT), 6 tiles
  grouped into one 2-bank PSUM tile; one batched exp (ACT) per group with
  a gap-skipping 4D access pattern; one tensor_reduce (DVE) for Z; one
  reciprocal; per-tile scalar-AP multiply -> s_norm bf16; one batched
  is_gt 0.5 -> s_ass; three accumulating matmuls (adj chunk [0:128,:],
  symmetric corner [28,28], sums via lhsT=[x,y,1]).
"""

import sys

sys.path.insert(0, "/opt/trn_rl_repo")

import numpy as np

N_CLUSTERS = 156
H = W = 1024
N_CORES = 8
ROWS_PER_CORE = H // N_CORES          # 128
CHUNK_ROWS = 16                        # image rows per SBUF im2col chunk
N_CHUNKS = ROWS_PER_CORE // CHUNK_ROWS  # 8
TILES_PER_CHUNK = CHUNK_ROWS * W // 128  # 128 pixel-tiles (128 px each)
PXW = W // 128                         # 8 column-blocks per row
# PSUM group: 6 tiles in 2 banks at offsets {0,156,312, 512,668,824}
GROUPS = [6] * 21 + [2]                # 21*6+2 = 128 tiles per chunk
G_OFF = [0, 156, 312, 512, 668, 824]

_CACHE = {}


def _build_graph():
    import concourse.bass as bass
    import concourse.bacc as bacc
    import concourse.tile as tile
    from concourse import mybir
    from contextlib import ExitStack

    f32 = mybir.dt.float32
    f32r = mybir.dt.float32r
    bf16 = mybir.dt.bfloat16
    AF = mybir.ActivationFunctionType

    nc = bacc.Bacc("TRN2")
    # xim[c, 18*m + k, u, p] = im2col row k of pixel-tile 3u+m, pixel p:
    # 3 tiles stacked on K so one [54,128]x[54,468] matmul (block-diagonal
    # weights) computes 3 convolutions at once.
    xim = nc.dram_tensor(
        "xim", (N_CHUNKS, 54, 43, 128), f32r, kind="ExternalInput"
    )
    # xy[p, r, cb, :] = (x, y, 1) of pixel (r, cb*128 + p), bf16
    xy = nc.dram_tensor("xy", (128, ROWS_PER_CORE, PXW, 3), bf16, kind="ExternalInput")
    w = nc.dram_tensor("w", (54, 3 * N_CLUSTERS), f32r, kind="ExternalInput")
    out = nc.dram_tensor("out", (160, N_CLUSTERS), f32, kind="ExternalOutput")

    with tile.TileContext(nc) as tc, ExitStack() as ctx:
        singles = ctx.enter_context(tc.tile_pool(name="singles", bufs=1))
        xpool = ctx.enter_context(tc.tile_pool(name="xpool", bufs=2))
        epool = ctx.enter_context(tc.tile_pool(name="epool", bufs=3))
        snpool = ctx.enter_context(tc.tile_pool(name="snpool", bufs=3))
        sapool = ctx.enter_context(tc.tile_pool(name="sapool", bufs=3))
        zpool = ctx.enter_context(tc.tile_pool(name="zpool", bufs=4))
        opool = ctx.enter_context(tc.tile_pool(name="opool", bufs=1))
        psc = ctx.enter_context(tc.tile_pool(name="psc", bufs=3, space="PSUM"))
        psacc = ctx.enter_context(tc.tile_pool(name="psacc", bufs=1, space="PSUM"))

        w_s = singles.tile([54, 3 * N_CLUSTERS], f32r)
        nc.sync.dma_start(out=w_s, in_=w[:, :])

        # persistent PSUM accumulators: adj rows 0:128, plus a combined
        # [sn2|xy1]^T @ [sn2|sa] matmul holding the adj corner and sums
        adj1_ps = psacc.tile([128, N_CLUSTERS], f32)
        combo_ps = psacc.tile([31, 184], f32)

        n_tiles_total = N_CHUNKS * TILES_PER_CHUNK

        xyf = xy.rearrange("p r cb ch -> p (r cb) ch")
        for c in range(N_CHUNKS):
            X = xpool.tile([54, 43, 128], f32r)
            nc.sync.dma_start(out=X, in_=xim[c])
            t0 = 0
            for g in GROUPS:
                tiles = list(range(t0, t0 + g))
                # --- conv: one 3-tile matmul per PSUM bank ---
                cps = psc.tile([128, 980], f32)
                if g == 6:
                    for h in range(2):
                        nc.tensor.matmul(
                            cps[:, 512 * h : 512 * h + 468],
                            lhsT=X[:, t0 // 3 + h, :],
                            rhs=w_s[:, :],
                            start=True,
                            stop=True,
                        )
                else:  # g == 2 tail: tiles 126,127 stacked as K=36
                    nc.tensor.matmul(
                        cps[:, 0:312],
                        lhsT=X[0:36, 42, :],
                        rhs=w_s[0:36, 0:312],
                        start=True,
                        stop=True,
                    )
                # --- batched exp, one op per PSUM bank (starts DVE earlier) ---
                e6 = epool.tile([128, 6, N_CLUSTERS], bf16)
                nb = (g + 2) // 3  # banks used (1 for g<=3, 2 for g=6)
                for b in range(nb):
                    nsl = min(g - 3 * b, 3)
                    src = bass.AP(
                        tensor=cps.tensor,
                        offset=cps.offset + 512 * b,
                        ap=[cps.ap[0], [156, nsl], [1, N_CLUSTERS]],
                    )
                    nc.scalar.activation(
                        out=e6[:, 3 * b : 3 * b + nsl, :], in_=src, func=AF.Exp
                    )
                # --- Z, 1/Z ---
                zg = zpool.tile([128, 6], f32)
                rg = zpool.tile([128, 6], f32)
                nc.vector.tensor_reduce(
                    out=zg[:, 0:g],
                    in_=e6[:, 0:g, :],
                    axis=mybir.AxisListType.X,
                    op=mybir.AluOpType.add,
                )
                nc.vector.reciprocal(out=rg[:, 0:g], in_=zg[:, 0:g])
                # --- normalize (per-tile scalar-AP mul) + threshold ---
                # sn6 cols 156:159 hold (x,y,1) so lhsT [sn2|xy1] is one AP
                sn6 = snpool.tile([128, 6, 160], bf16)
                nc.sync.dma_start(
                    out=sn6[:, 0:g, 156:159],
                    in_=xyf[:, c * TILES_PER_CHUNK + t0 : c * TILES_PER_CHUNK + t0 + g, :],
                )
                for j in range(g):
                    nc.vector.tensor_scalar_mul(
                        sn6[:, j, 0:N_CLUSTERS], e6[:, j, :], rg[:, j : j + 1]
                    )
                # sab = [sn[:,128:156] | s_ass] so the combined rhs is one AP
                sab = sapool.tile([128, 6, 184], bf16)
                nc.vector.tensor_copy(
                    out=sab[:, 0:g, 0:28], in_=sn6[:, 0:g, 128:156]
                )
                nc.vector.tensor_scalar(
                    out=sab[:, 0:g, 28:184],
                    in0=sn6[:, 0:g, 0:N_CLUSTERS],
                    scalar1=0.5,
                    scalar2=None,
                    op0=mybir.AluOpType.is_gt,
                )
                # --- accumulating matmuls (same-target back-to-back) ---
                for j, t in enumerate(tiles):
                    first = c == 0 and t0 == 0 and j == 0
                    last = c == N_CHUNKS - 1 and t0 + g == TILES_PER_CHUNK and j == g - 1
                    nc.tensor.matmul(
                        adj1_ps,
                        lhsT=sn6[:, j, 0:128],
                        rhs=sn6[:, j, 0:N_CLUSTERS],
                        start=first,
                        stop=last,
                    )
                for j, t in enumerate(tiles):
                    first = c == 0 and t0 == 0 and j == 0
                    last = c == N_CHUNKS - 1 and t0 + g == TILES_PER_CHUNK and j == g - 1
                    nc.tensor.matmul(
                        combo_ps,
                        lhsT=sn6[:, j, 128:159],
                        rhs=sab[:, j, :],
                        start=first,
                        stop=last,
                    )
                t0 += g

        o1 = opool.tile([128, N_CLUSTERS], f32)
        nc.vector.tensor_copy(out=o1, in_=adj1_ps)
        nc.sync.dma_start(out=out[0:128, :], in_=o1)
        o23 = opool.tile([31, 184], f32)
        nc.vector.tensor_copy(out=o23, in_=combo_ps)
        nc.sync.dma_start(out=out[128:131, :], in_=o23[28:31, 28:184])
        nc.sync.dma_start(out=out[131:159, 0:28], in_=o23[0:28, 0:28])

    nc.compile()
    return nc


def _get_graph():
    if "nc" not in _CACHE:
        _CACHE["nc"] = _build_graph()
    return _CACHE["nc"]


def _prep_inputs(inputs, conv_w, conv_b):
    img = np.asarray(inputs, np.float32).reshape(H, W, 2)
    padded = np.zeros((2, H + 2, W + 2), np.float32)
    padded[:, 1 : H + 1, 1 : W + 1] = img.transpose(2, 0, 1)
    # host-side im2col: big[i, c, k, r, col] = padded[ch, i*128 + c*16 + kr + r, kc + col]
    big = np.empty((N_CORES, N_CHUNKS, 18, CHUNK_ROWS, W), np.float32)
    for kc in range(3):
        for kr in range(3):
            for ch in range(2):
                k = kc * 6 + kr * 2 + ch
                v = padded[ch, kr : kr + H, kc : kc + W]
                big[:, :, k] = v.reshape(N_CORES, N_CHUNKS, CHUNK_ROWS, W)
    # stack 3 consecutive pixel-tiles on the contraction dim (K=54)
    bigt = big.reshape(N_CORES, N_CHUNKS, 18, TILES_PER_CHUNK, 128)
    xim54 = np.zeros((N_CORES, N_CHUNKS, 54, 43, 128), np.float32)
    for m in range(3):
        idx = np.arange(42) * 3 + m
        xim54[:, :, 18 * m : 18 * (m + 1), 0:42] = bigt[:, :, :, idx]
        if m < 2:
            xim54[:, :, 18 * m : 18 * (m + 1), 42] = bigt[:, :, :, 126 + m]
    # constant conv bias is softmax-invariant; drop it.
    # k = kc*6 + kr*2 + ch  ->  conv_w[kr, kc, ch, :]; block-diagonal x3
    w18 = np.asarray(conv_w, np.float32).transpose(1, 0, 2, 3).reshape(18, N_CLUSTERS)
    w19 = np.zeros((54, 3 * N_CLUSTERS), np.float32)
    for m in range(3):
        w19[18 * m : 18 * (m + 1), N_CLUSTERS * m : N_CLUSTERS * (m + 1)] = w18
    import ml_dtypes

    bf = ml_dtypes.bfloat16
    in_maps = []
    for i in range(N_CORES):
        r0 = i * ROWS_PER_CORE
        xyi = np.ones((128, ROWS_PER_CORE, PXW, 3), np.float32)
        xyi[:, :, :, 0:2] = (
            img[r0 : r0 + ROWS_PER_CORE]
            .reshape(ROWS_PER_CORE, PXW, 128, 2)
            .transpose(2, 0, 1, 3)
        )
        in_maps.append({"xim": xim54[i], "xy": xyi.astype(bf), "w": w19})
    return in_maps


def _postprocess(outs):
    o = np.zeros((160, N_CLUSTERS), np.float64)
    for r in outs:
        o += np.asarray(r["out"], np.float64)
    adj = np.zeros((N_CLUSTERS, N_CLUSTERS), np.float64)
    adj[0:128, :] = o[0:128]
    adj[128:156, 128:156] = o[131:159, 0:28]
    adj[128:156, 0:128] = o[0:128, 128:156].T
    sums = o[128:130].T          # [156, 2]
    counts = o[130]              # [156]
    with np.errstate(divide="ignore", invalid="ignore"):
        nodes = sums / counts[:, None]
    return nodes.astype(np.float32), adj.astype(np.float32)


def _install_ntff_hook():
    """The agent image's antenv lacks axon_hooks; synthesize it and register
    the ctypes NTFF profiling hook against the loaded libaxon_pjrt.so."""
    import sys as _sys
    import types, ctypes, contextlib

    if "antenv.axon_hooks" in _sys.modules:
        return
    mod = types.ModuleType("antenv.axon_hooks")
    state = {"hook": None}
    mod.set_axon_ntff_profile_hook = lambda h: state.__setitem__("hook", h)
    mod.get_axon_ntff_profile_hook = lambda: state["hook"]
    _sys.modules["antenv.axon_hooks"] = mod
    import antenv

    antenv.axon_hooks = mod

    so_path = "/opt/axon/libaxon_pjrt.so"
    lib = ctypes.CDLL(so_path)
    if not hasattr(lib, "axon_start_nrt_profile"):
        return
    lib.axon_start_nrt_profile.argtypes = [
        ctypes.POINTER(ctypes.c_int64),
        ctypes.c_size_t,
    ]
    lib.axon_start_nrt_profile.restype = ctypes.c_int64
    lib.axon_stop_nrt_profile.argtypes = [ctypes.c_char_p]
    lib.axon_stop_nrt_profile.restype = ctypes.c_int64

    @contextlib.contextmanager
    def _hook(output_dir, device_ids):
        import jax

        jax.devices()
        if device_ids:
            ids = (ctypes.c_int64 * len(device_ids))(*device_ids)
            rc = lib.axon_start_nrt_profile(ids, len(device_ids))
        else:
            rc = lib.axon_start_nrt_profile(None, 0)
        if rc != 0:
            raise RuntimeError(f"axon_start_nrt_profile rc={rc}")
        try:
            yield
        finally:
            n = lib.axon_stop_nrt_profile(str(output_dir).encode())
            print(f"profile: {n} file(s) written to {output_dir}", file=_sys.stderr)

    mod.set_axon_ntff_profile_hook(_hook)


def _run(inputs, conv_w, conv_b, trace=False, **trace_kwargs):
    from concourse.bass_utils import run_bass_kernel_spmd

    if trace:
        _install_ntff_hook()

    nc = _get_graph()
    in_maps = _prep_inputs(inputs, conv_w, conv_b)
    res = run_bass_kernel_spmd(
        nc, in_maps, core_ids=list(range(N_CORES)), trace=trace, **trace_kwargs
    )
    return _postprocess(res.results), res


def kernel(inputs, conv_w, conv_b):
    (nodes, adj), _ = _run(inputs, conv_w, conv_b)
    return nodes, adj
